# revision 9
# baseline (speedup 1.0000x reference)
"""Trainium2 Bass kernel for the consistency-loss problem.

loss = -mean_b( table[argmax_c pred1[b,c]] . log_softmax(pred2[b]) )

Fast path ("fast2") exploits the block structure of the harness table
(table[c, c*10:(c+1)*10] = u, zeros elsewhere, same u for every row):

    loss_b = BLOCK*u * lse_b - u * S[b, c*_b]
    lse_b  = log(sum_j exp(pred2[b,j]))
    S[b,c] = sum of the 10-wide block c of pred2 row b

Per 128x1000 segment the device computes only two row scalars:
 - se  = sum_j exp(pred2)       (ACT Exp, row-sum on the ACT accumulator
                                 or a fused Pool op)
 - dot = sum_j pred2 * mask     (one fused multiply+row-reduce
                                 scalar_tensor_tensor; mask is the argmax
                                 one-hot broadcast 10-wide via a stride-0
                                 view, so no PE matmul and no table on
                                 device at all)
Both ship as one [128, 128] tile; the host applies log/scales in f64.

Engine budget per core (64 segments, ~86us DMA stream @ ~425 GB/s):
 - ACT: 64 Exp (~69us) + 40 accumulator reads (~10us)
 - Pool (otherwise idle!): 24 se row-sums + 12 dot jobs (~56us)
 - DVE: one-hot quarters (~14us) + 52 dot jobs (~61us)
 - PE: unused.  All engines < DMA stream -> DMA-bound.
DMA: tile0 + pred1 ride the SWDGE (gpsimd) queue starting ~3us, before
the sync-ring preamble finishes (~8us); tiles 1..15 own the sync ring.
Tail tiles split finer so the last segments land continuously.

Sharding: data-parallel over B across 8 NeuronCores; host combines the
per-core partial outputs.  Non-block tables fall back to the previous
matmul-based programs ("fast"/"general"), which handle any table.
"""

import sys
from contextlib import ExitStack

import numpy as np

for _p in ("/opt/trn_rl_repo", "/root/.axon_site/_ro/trn_rl_repo"):
    if _p not in sys.path:
        sys.path.append(_p)

import concourse.bass as bass
import concourse.tile as tile
from concourse import bacc, mybir
from concourse.bass_utils import run_bass_kernel_spmd

B, C1, C2 = 65536, 100, 1000
BLOCK = C2 // C1            # 10 fine classes per coarse class
NCORES = 8
BC = B // NCORES            # rows per core (8192)
P = 128                     # partitions
KS = 4                      # sub-rows per partition per tile
NT = BC // (P * KS)         # tiles per core (16)
NSEG = BC // P              # per-row segments per core (64) == NT*KS
NQ = 4                      # one-hot quarters (DVE op granularity)
JQ = NSEG // NQ             # segments per quarter (16)
F32 = mybir.dt.float32
F32R = mybir.dt.float32r
X = mybir.AxisListType.X
ALU = mybir.AluOpType
ACTF = mybir.ActivationFunctionType

# ---- fast2 static knobs ----------------------------------------------------
# walrus only lowers a narrow op set on Pool; is_ge one-hot quarters go
# there if accepted (saves ~7us of DVE), else stay on the DVE.
POOL_ISGE = False
# tiles whose dot job runs per-segment (overlaps the k-split tail DMAs)
SPLIT_D_TILES = frozenset({NT - 1})
# tiles whose exp row-sums reduce on the DVE instead of the ACT
# accumulator (relieves the ACT critical path; DVE idles early anyway)
DVE_R_TILES = frozenset()

# dot-column layout: one output column per D instruction (merged tiles get
# one, split tiles get KS); the host sums them all, so no gaps allowed.
_D_COLS: dict = {}
for _i in range(NT):
    if _i in SPLIT_D_TILES:
        for _k in range(KS):
            _D_COLS[(_i, _k)] = len(_D_COLS)
    else:
        _D_COLS[(_i, None)] = len(_D_COLS)
ND = len(_D_COLS)


def _build_fast2() -> bass.Bass:
    nc = bacc.Bacc("TRN2", target_bir_lowering=False, debug=False,
                   num_devices=NCORES)
    p1 = nc.dram_tensor("p1", [BC, C1], F32, kind="ExternalInput").ap()
    p2 = nc.dram_tensor("p2", [BC, C2], F32, kind="ExternalInput").ap()
    sd_out = nc.dram_tensor("sd", [P, NSEG + ND], F32,
                            kind="ExternalOutput").ap()

    with tile.TileContext(nc) as tc:
        with ExitStack() as ctx:
            _fast2_body(ctx, tc, p1, p2, sd_out)
    nc.compile()
    return nc


def _fast2_body(ctx: ExitStack, tc, p1, p2, sd_out):
    nc = tc.nc
    consts = ctx.enter_context(tc.tile_pool(name="consts", bufs=1))
    p2pool = ctx.enter_context(tc.tile_pool(name="p2", bufs=5))
    etp = ctx.enter_context(tc.tile_pool(name="expp", bufs=2))

    # row (p*64 + i*4 + k)  <->  tile i, partition p, sub-row k
    p2t = p2.rearrange("(p i k) c -> i p (k c)", p=P, i=NT, k=KS)
    # pred1 halves: per-partition 12.8KB contiguous runs
    p1h = p1.rearrange("(p h j) c -> h p (j c)", p=P, h=2, j=NSEG // 2)

    p1big = consts.tile([P, NSEG * C1], F32)
    oh_all = consts.tile([P, NSEG * C1], F32)
    rmax = consts.tile([P, NSEG], F32)
    sd_all = consts.tile([P, NSEG + ND], F32)
    scr = consts.tile([P, KS * C2], F32)

    p1big3 = p1big[:].rearrange("p (j c) -> p j c", j=NSEG)
    oh3 = oh_all[:].rearrange("p (j c) -> p j c", j=NSEG)

    def onehot_quarter(q):
        js = slice(q * JQ, (q + 1) * JQ)
        nc.vector.reduce_max(rmax[:, js], p1big3[:, js, :], axis=X)
        rmx3 = rmax[:, js].unsqueeze(2).broadcast_to((P, JQ, C1))
        eng = nc.gpsimd if POOL_ISGE else nc.vector
        eng.tensor_tensor(oh3[:, js, :], p1big3[:, js, :], rmx3,
                          op=ALU.is_ge)

    t2_tiles = []

    def load_tile(i, eng):
        t2 = p2pool.tile([P, KS * C2], F32, tag="p2")
        if i == 0 or i == NT - 1:
            # k-split: pipeline head starts early / tail lands continuously
            for k in range(KS):
                eng.dma_start(t2[:, bass.ts(k, C2)], p2t[i][:, bass.ts(k, C2)])
        elif i >= NT - 3:
            # pair-split taper for the tiles feeding the tail
            for k in range(0, KS, 2):
                eng.dma_start(t2[:, k * C2:(k + 2) * C2],
                              p2t[i][:, k * C2:(k + 2) * C2])
        else:
            eng.dma_start(t2[:], p2t[i])
        t2_tiles.append(t2)

    def consume_tile(i):
        t2 = t2_tiles[i]
        # dot job(s): fused (pred2 * blockmask) + row-reduce in a single DVE
        # instruction.  The dot term is linear, so a whole tile can fold
        # into one accumulator column; split tiles go per-segment so the
        # tail overlaps their k-split DMAs.
        if i in SPLIT_D_TILES:
            for k in range(KS):
                seg = i * KS + k
                t2s = t2[:, bass.ts(k, C2)].rearrange(
                    "p (c b) -> p c b", b=BLOCK)
                scr3 = scr[:, bass.ts(k, C2)].rearrange(
                    "p (c b) -> p c b", b=BLOCK)
                mask3 = oh3[:, seg, :].unsqueeze(2).broadcast_to(
                    (P, C1, BLOCK))
                dcol = NSEG + _D_COLS[(i, k)]
                nc.vector.scalar_tensor_tensor(
                    scr3, t2s, 1.0, mask3, op0=ALU.mult, op1=ALU.mult,
                    accum_out=sd_all[:, dcol:dcol + 1])
        else:
            t2v = t2[:].rearrange("p (s c b) -> p s c b", s=KS, b=BLOCK)
            scr4 = scr[:].rearrange("p (s c b) -> p s c b", s=KS, b=BLOCK)
            mask4 = oh3[:, i * KS:(i + 1) * KS, :].unsqueeze(3).broadcast_to(
                (P, KS, C1, BLOCK))
            dcol = NSEG + _D_COLS[(i, None)]
            nc.vector.scalar_tensor_tensor(
                scr4, t2v, 1.0, mask4, op0=ALU.mult, op1=ALU.mult,
                accum_out=sd_all[:, dcol:dcol + 1])
        for k in range(KS):
            seg = i * KS + k
            t2f = t2[:, bass.ts(k, C2)]
            se_col = sd_all[:, seg:seg + 1]
            et = etp.tile([P, C2], F32, tag="exp1")
            if i in DVE_R_TILES:
                nc.scalar.activation(et[:], t2f, ACTF.Exp)
                nc.vector.reduce_sum(se_col, et[:], axis=X)
            else:
                nc.scalar.activation(et[:], t2f, ACTF.Exp, accum_out=se_col)

    # --- DMA schedule ---
    # SWDGE (gpsimd ring) starts generating descriptors ~5us before the
    # sync-ring preamble finishes: tile0 + pred1 ride it so compute and the
    # one-hot can start early.  Tiles 1..15 own the sync ring.
    t0 = p2pool.tile([P, KS * C2], F32, tag="p2")
    t2_tiles.append(t0)
    nc.gpsimd.dma_start(t0[:, 0:2 * C2], p2t[0][:, 0:2 * C2])
    nc.gpsimd.dma_start(p1big[:, 0:NSEG * C1 // 2], p1h[0])
    nc.gpsimd.dma_start(t0[:, 2 * C2:], p2t[0][:, 2 * C2:])
    nc.gpsimd.dma_start(p1big[:, NSEG * C1 // 2:], p1h[1])
    for i in range(1, NT):
        load_tile(i, nc.sync)

    for i in range(NT):
        if i % 4 == 0:
            onehot_quarter(i // 4)
        consume_tile(i)

    nc.sync.dma_start(sd_out[:, :], sd_all[:])


# ===========================================================================
# Fallback programs (arbitrary tables): previous matmul-based kernel.
# ===========================================================================

CHUNKS = [(0, 512), (512, C2)]
GA_LAST = NT - 4
ACT_ACCUM_SEGS = frozenset(s for s in range(4, 56, 3)) | {NSEG - 2, NSEG - 1}


def _build_program(general: bool) -> bass.Bass:
    nc = bacc.Bacc("TRN2", target_bir_lowering=False, debug=False,
                   num_devices=NCORES)
    p1 = nc.dram_tensor("p1", [BC, C1], F32, kind="ExternalInput").ap()
    p2 = nc.dram_tensor("p2", [BC, C2], F32, kind="ExternalInput").ap()
    tbl = nc.dram_tensor("tbl", [C1, C2], F32, kind="ExternalInput").ap()
    sbc = None
    if general:
        sbc = nc.dram_tensor("sbc", [P, C1], F32, kind="ExternalInput").ap()
    se_out = nc.dram_tensor("se", [P, NSEG], F32, kind="ExternalOutput").ap()
    rd_out = nc.dram_tensor("rd", [C1, 2], F32, kind="ExternalOutput").ap()
    sel_out = None
    if general:
        sel_out = nc.dram_tensor("sel", [P, NSEG], F32,
                                 kind="ExternalOutput").ap()

    with tile.TileContext(nc) as tc:
        with ExitStack() as ctx:
            _kernel_body(ctx, tc, p1, p2, tbl, sbc, se_out, rd_out, sel_out,
                         general)
    nc.compile()
    return nc


def _kernel_body(ctx: ExitStack, tc, p1, p2, tbl, sbc, se_out, rd_out,
                 sel_out, general):
    nc = tc.nc
    consts = ctx.enter_context(tc.tile_pool(name="consts", bufs=1))
    p2pool = ctx.enter_context(tc.tile_pool(name="p2", bufs=5))
    expp = ctx.enter_context(tc.tile_pool(name="expp", bufs=6))
    psum = ctx.enter_context(tc.tile_pool(name="psum", bufs=1, space="PSUM"))

    p2t = p2.rearrange("(p i k) c -> i p (k c)", p=P, i=NT, k=KS)
    p1h = p1.rearrange("(p h j) c -> h p (j c)", p=P, h=2, j=NSEG // 2)

    p1big = consts.tile([P, NSEG * C1], F32)
    oh_all = consts.tile([P, NSEG * C1], F32R)
    tbl_sb = consts.tile([C1, C2], F32)
    rmax = consts.tile([P, NSEG], F32)
    se_all = consts.tile([P, NSEG], F32)
    rowdots = consts.tile([C1, 2], F32)

    if general:
        sbc_sb = consts.tile([P, C1], F32)
        ss_scratch = consts.tile([P, JQ * C1], F32)
        ss3 = ss_scratch[:].rearrange("p (j c) -> p j c", j=JQ)
        sbc3 = sbc_sb[:].unsqueeze(1).broadcast_to((P, JQ, C1))
        sel_s_all = consts.tile([P, NSEG], F32)

    G_a = psum.tile([C1, C2], F32)
    G_b = psum.tile([C1, C2], F32)

    p1big3 = p1big[:].rearrange("p (j c) -> p j c", j=NSEG)
    oh3 = oh_all[:].rearrange("p (j c) -> p j c", j=NSEG)

    def onehot_quarter(q):
        js = slice(q * JQ, (q + 1) * JQ)
        nc.vector.reduce_max(rmax[:, js], p1big3[:, js, :], axis=X)
        rmx3 = rmax[:, js].unsqueeze(2).broadcast_to((P, JQ, C1))
        nc.vector.tensor_tensor(oh3[:, js, :], p1big3[:, js, :], rmx3,
                                op=ALU.is_ge)
        if general:
            nc.vector.tensor_tensor(ss3[:], oh3[:, js, :].bitcast(F32), sbc3,
                                    op=ALU.mult)
            nc.vector.reduce_sum(sel_s_all[:, js], ss3[:], axis=X)

    t2_tiles = []

    def load_tile(i):
        t2 = p2pool.tile([P, KS * C2], F32R, tag="p2")
        if i == 0 or i == NT - 1:
            for k in range(KS):
                nc.sync.dma_start(t2[:, bass.ts(k, C2)],
                                  p2t[i][:, bass.ts(k, C2)].bitcast(F32R))
        elif i >= NT - 3:
            for k in range(0, KS, 2):
                nc.sync.dma_start(t2[:, k * C2:(k + 2) * C2],
                                  p2t[i][:, k * C2:(k + 2) * C2].bitcast(F32R))
        else:
            nc.sync.dma_start(t2[:], p2t[i].bitcast(F32R))
        t2_tiles.append(t2)

    def consume_tile(i):
        t2 = t2_tiles[i]
        for k in range(KS):
            seg = i * KS + k
            se_col = se_all[:, seg:seg + 1]
            et = expp.tile([P, C2], F32, tag="exp1")
            if seg in ACT_ACCUM_SEGS:
                nc.scalar.activation(et[:], t2[:, bass.ts(k, C2)].bitcast(F32),
                                     ACTF.Exp, accum_out=se_col)
            else:
                nc.scalar.activation(et[:], t2[:, bass.ts(k, C2)].bitcast(F32),
                                     ACTF.Exp)
                nc.vector.reduce_sum(se_col, et[:], axis=X)
        G = G_a if i <= GA_LAST else G_b
        for k in range(KS):
            seg = i * KS + k
            for lo, hi in CHUNKS:
                nc.tensor.matmul(G[:, lo:hi], oh_all[:, bass.ts(seg, C1)],
                                 t2[:, k * C2 + lo:k * C2 + hi],
                                 start=(k == 0 and i in (0, GA_LAST + 1)),
                                 stop=(k == KS - 1 and i in (GA_LAST, NT - 1)))

    nc.sync.dma_start(p1big[:, 0:NSEG * C1 // 2], p1h[0])
    load_tile(0)
    load_tile(1)
    nc.sync.dma_start(p1big[:, NSEG * C1 // 2:], p1h[1])
    load_tile(2)
    nc.sync.dma_start(tbl_sb[:], tbl[:, :])
    if general:
        nc.sync.dma_start(sbc_sb[:], sbc[:, :])
    for i in range(3, NT):
        load_tile(i)

    gt_scratch = consts.tile([C1, C2], F32)
    for i in range(NT):
        if i % 4 == 0:
            onehot_quarter(i // 4)
        consume_tile(i)
        if i == GA_LAST + 1:
            nc.vector.tensor_mul(gt_scratch[:], G_a[:], tbl_sb[:])
            nc.vector.tensor_reduce(rowdots[:, 0:1], gt_scratch[:], axis=X,
                                    op=ALU.add, negate=True)

    nc.vector.tensor_mul(gt_scratch[:], G_b[:], tbl_sb[:])
    nc.vector.tensor_reduce(rowdots[:, 1:2], gt_scratch[:], axis=X,
                            op=ALU.add, negate=True)
    nc.sync.dma_start(se_out[:, :], se_all[:])
    if general:
        nc.sync.dma_start(sel_out[:, :], sel_s_all[:])
    nc.sync.dma_start(rd_out[:, :], rowdots[:])


_PROGRAM_CACHE: dict = {}


def _program(mode: str = "fast2") -> bass.Bass:
    if mode not in _PROGRAM_CACHE:
        if mode == "fast2":
            _PROGRAM_CACHE[mode] = _build_fast2()
        else:
            _PROGRAM_CACHE[mode] = _build_program(mode == "general")
    return _PROGRAM_CACHE[mode]


def _row_sums(table):
    return np.asarray(table, dtype=np.float32).sum(axis=1, dtype=np.float32)


def _is_uniform_s(s):
    return bool(np.all(np.abs(s - s[0]) <= 1e-6 * max(1.0, abs(float(s[0])))))


def _block_uniform_u(table):
    """u if table is the uniform block table (t[c, c*B:(c+1)*B] = u), else
    None."""
    t = np.ascontiguousarray(table, dtype=np.float32)
    if t.shape != (C1, C2):
        return None
    u = float(t[0, 0])
    if u == 0.0:
        return None
    expect = np.zeros((C1, C2), dtype=np.float32)
    for c in range(C1):
        expect[c, c * BLOCK:(c + 1) * BLOCK] = np.float32(u)
    return u if np.array_equal(t, expect) else None


def _in_maps(pred1_logits, pred2_logits, table, mode: str):
    p1 = np.ascontiguousarray(pred1_logits, dtype=np.float32)
    p2 = np.ascontiguousarray(pred2_logits, dtype=np.float32)
    tbl = np.ascontiguousarray(table, dtype=np.float32)
    maps = []
    for k in range(NCORES):
        m = {
            "p1": np.ascontiguousarray(p1[k * BC:(k + 1) * BC]),
            "p2": np.ascontiguousarray(p2[k * BC:(k + 1) * BC]),
        }
        if mode != "fast2":
            m["tbl"] = tbl
        if mode == "general":
            s = _row_sums(tbl)
            m["sbc"] = np.ascontiguousarray(np.tile(s, (P, 1)))
        maps.append(m)
    return maps


def _combine_fast2(result, u):
    sd = np.asarray(result["sd"], dtype=np.float64)
    se = sd[:, :NSEG]
    dot = sd[:, NSEG:NSEG + ND]
    return (BLOCK * u) * np.log(se).sum() - u * dot.sum()


def _combine(result, s0, general):
    lse = np.log(np.asarray(result["se"], dtype=np.float64))
    if general:
        lse = lse * np.asarray(result["sel"], dtype=np.float64)
        lse_term = lse.sum()
    else:
        lse_term = s0 * lse.sum()
    return lse_term + np.asarray(result["rd"], dtype=np.float64).sum()


def run_on_device(pred1_logits, pred2_logits, table, **spmd_kwargs):
    """Compile/run the SPMD program on cores 0-7; returns (loss, results)."""
    u = _block_uniform_u(table)
    if u is not None:
        mode = "fast2"
    else:
        s = _row_sums(table)
        mode = "fast" if _is_uniform_s(s) else "general"
    nc = _program(mode)
    res = run_bass_kernel_spmd(
        nc, _in_maps(pred1_logits, pred2_logits, table, mode),
        core_ids=list(range(NCORES)), **spmd_kwargs)
    if mode == "fast2":
        partials = [_combine_fast2(r, u) for r in res.results]
    else:
        s0 = np.float64(_row_sums(table)[0])
        partials = [_combine(r, s0, mode == "general") for r in res.results]
    loss = np.float32(np.sum(partials, dtype=np.float64) / B)
    return np.asarray(loss), res


def kernel(pred1_logits, pred2_logits, table):
    loss, _ = run_on_device(pred1_logits, pred2_logits, table)
    return loss


# revision 11
# speedup vs baseline: 1.0685x; 1.0685x over previous
"""Trainium2 Bass kernel for the consistency-loss problem.

loss = -mean_b( table[argmax_c pred1[b,c]] . log_softmax(pred2[b]) )

Fast path ("fast2") exploits the block structure of the harness table
(table[c, c*10:(c+1)*10] = u, zeros elsewhere, same u for every row):

    loss_b = BLOCK*u * lse_b - u * S[b, c*_b]
    lse_b  = log(sum_j exp(pred2[b,j]))
    S[b,c] = sum of the 10-wide block c of pred2 row b

Per 128x1000 segment the device computes only two row scalars:
 - se  = sum_j exp(pred2)       (ACT Exp, row-sum on the ACT accumulator
                                 or a fused Pool op)
 - dot = sum_j pred2 * mask     (one fused multiply+row-reduce
                                 scalar_tensor_tensor; mask is the argmax
                                 one-hot broadcast 10-wide via a stride-0
                                 view, so no PE matmul and no table on
                                 device at all)
Both ship as one [128, 128] tile; the host applies log/scales in f64.

Engine budget per core (64 segments, ~86us DMA stream @ ~425 GB/s):
 - ACT: 64 Exp (~69us) + 40 accumulator reads (~10us)
 - Pool (otherwise idle!): 24 se row-sums + 12 dot jobs (~56us)
 - DVE: one-hot quarters (~14us) + 52 dot jobs (~61us)
 - PE: unused.  All engines < DMA stream -> DMA-bound.
DMA: tile0 + pred1 ride the SWDGE (gpsimd) queue starting ~3us, before
the sync-ring preamble finishes (~8us); tiles 1..15 own the sync ring.
Tail tiles split finer so the last segments land continuously.

Sharding: data-parallel over B across 8 NeuronCores; host combines the
per-core partial outputs.  Non-block tables fall back to the previous
matmul-based programs ("fast"/"general"), which handle any table.
"""

import sys
from contextlib import ExitStack

import numpy as np

for _p in ("/opt/trn_rl_repo", "/root/.axon_site/_ro/trn_rl_repo"):
    if _p not in sys.path:
        sys.path.append(_p)

import concourse.bass as bass
import concourse.tile as tile
from concourse import bacc, mybir
from concourse.bass_utils import run_bass_kernel_spmd

B, C1, C2 = 65536, 100, 1000
BLOCK = C2 // C1            # 10 fine classes per coarse class
NCORES = 8
BC = B // NCORES            # rows per core (8192)
P = 128                     # partitions
KS = 4                      # sub-rows per partition per tile
NT = BC // (P * KS)         # tiles per core (16)
NSEG = BC // P              # per-row segments per core (64) == NT*KS
NQ = 4                      # one-hot quarters (DVE op granularity)
JQ = NSEG // NQ             # segments per quarter (16)
F32 = mybir.dt.float32
F32R = mybir.dt.float32r
X = mybir.AxisListType.X
ALU = mybir.AluOpType
ACTF = mybir.ActivationFunctionType

# ---- fast2 static knobs ----------------------------------------------------
# dot-term columns: one per segment (fused is_ge+mult+row-reduce per seg)
ND = NSEG


def _build_fast2() -> bass.Bass:
    nc = bacc.Bacc("TRN2", target_bir_lowering=False, debug=False,
                   num_devices=NCORES)
    p1 = nc.dram_tensor("p1", [BC, C1], F32, kind="ExternalInput").ap()
    p2 = nc.dram_tensor("p2", [BC, C2], F32, kind="ExternalInput").ap()
    sd_out = nc.dram_tensor("sd", [P, NSEG + ND], F32,
                            kind="ExternalOutput").ap()

    with tile.TileContext(nc) as tc:
        with ExitStack() as ctx:
            _fast2_body(ctx, tc, p1, p2, sd_out)
    nc.compile()
    return nc


def _fast2_body(ctx: ExitStack, tc, p1, p2, sd_out):
    nc = tc.nc
    consts = ctx.enter_context(tc.tile_pool(name="consts", bufs=1))
    p2pool = ctx.enter_context(tc.tile_pool(name="p2", bufs=6))
    etp = ctx.enter_context(tc.tile_pool(name="expp", bufs=2))

    # row (p*64 + i*4 + k)  <->  tile i, partition p, sub-row k
    p2t = p2.rearrange("(p i k) c -> i p (k c)", p=P, i=NT, k=KS)
    # pred1 quarters: per-partition 6.4KB contiguous runs, arriving just
    # ahead of the segments whose row-max they feed
    p1q = p1.rearrange("(p q j) c -> q p (j c)", p=P, q=NQ, j=JQ)

    p1big = consts.tile([P, NSEG * C1], F32)
    rmax = consts.tile([P, NSEG], F32)
    sd_all = consts.tile([P, NSEG + ND], F32)
    scr = consts.tile([P, C2], F32)

    p1big3 = p1big[:].rearrange("p (j c) -> p j c", j=NSEG)
    scr3 = scr[:].rearrange("p (c b) -> p c b", b=BLOCK)

    def rmax_quarter(q):
        js = slice(q * JQ, (q + 1) * JQ)
        nc.vector.reduce_max(rmax[:, js], p1big3[:, js, :], axis=X)

    t2_tiles = []

    def load_tile(i, eng):
        t2 = p2pool.tile([P, KS * C2], F32, tag="p2")
        if i == 0 or i == NT - 1:
            # k-split: pipeline head starts early / tail lands continuously
            for k in range(KS):
                eng.dma_start(t2[:, bass.ts(k, C2)], p2t[i][:, bass.ts(k, C2)])
        elif i >= NT - 3:
            # pair-split taper for the tiles feeding the tail
            for k in range(0, KS, 2):
                eng.dma_start(t2[:, k * C2:(k + 2) * C2],
                              p2t[i][:, k * C2:(k + 2) * C2])
        else:
            eng.dma_start(t2[:], p2t[i])
        t2_tiles.append(t2)

    def consume_tile(i):
        t2 = t2_tiles[i]
        for k in range(KS):
            seg = i * KS + k
            t2f = t2[:, bass.ts(k, C2)]
            se_col = sd_all[:, seg:seg + 1]
            dot_col = sd_all[:, NSEG + seg:NSEG + seg + 1]
            # fused dot job on the DVE: (pred1_bcast >= rmax) * pred2,
            # row-reduced into the accumulator — the one-hot never
            # materializes and the whole dot term is 1 instr / segment.
            p1b = p1big3[:, seg, :].unsqueeze(2).broadcast_to((P, C1, BLOCK))
            t2s = t2f.rearrange("p (c b) -> p c b", b=BLOCK)
            nc.vector.scalar_tensor_tensor(
                scr3, p1b, rmax[:, seg:seg + 1], t2s,
                op0=ALU.is_ge, op1=ALU.mult, accum_out=dot_col)
            # exp + row-sum on the ACT accumulator
            et = etp.tile([P, C2], F32, tag="exp1")
            nc.scalar.activation(et[:], t2f, ACTF.Exp, accum_out=se_col)

    # --- DMA schedule ---
    # ACT ring (HWDGE, ~2.5us preamble) carries what the head of the
    # pipeline needs: pred1 quarter 0, then tile0 in two halves.  The sync
    # ring carries everything else; pred1 quarters 1-3 drop in between the
    # early tiles so each lands well before its segments' dot jobs.
    t0 = p2pool.tile([P, KS * C2], F32, tag="p2")
    t2_tiles.append(t0)
    nc.scalar.dma_start(p1big[:, 0:JQ * C1], p1q[0])
    nc.scalar.dma_start(t0[:, 0:2 * C2], p2t[0][:, 0:2 * C2])
    nc.scalar.dma_start(t0[:, 2 * C2:], p2t[0][:, 2 * C2:])
    for i in range(1, NT):
        load_tile(i, nc.sync)
        if i <= 3:
            nc.sync.dma_start(p1big[:, i * JQ * C1:(i + 1) * JQ * C1], p1q[i])

    for i in range(NT):
        if i % 4 == 0:
            rmax_quarter(i // 4)
        consume_tile(i)

    nc.sync.dma_start(sd_out[:, :], sd_all[:])


# ===========================================================================
# Fallback programs (arbitrary tables): previous matmul-based kernel.
# ===========================================================================

CHUNKS = [(0, 512), (512, C2)]
GA_LAST = NT - 4
ACT_ACCUM_SEGS = frozenset(s for s in range(4, 56, 3)) | {NSEG - 2, NSEG - 1}


def _build_program(general: bool) -> bass.Bass:
    nc = bacc.Bacc("TRN2", target_bir_lowering=False, debug=False,
                   num_devices=NCORES)
    p1 = nc.dram_tensor("p1", [BC, C1], F32, kind="ExternalInput").ap()
    p2 = nc.dram_tensor("p2", [BC, C2], F32, kind="ExternalInput").ap()
    tbl = nc.dram_tensor("tbl", [C1, C2], F32, kind="ExternalInput").ap()
    sbc = None
    if general:
        sbc = nc.dram_tensor("sbc", [P, C1], F32, kind="ExternalInput").ap()
    se_out = nc.dram_tensor("se", [P, NSEG], F32, kind="ExternalOutput").ap()
    rd_out = nc.dram_tensor("rd", [C1, 2], F32, kind="ExternalOutput").ap()
    sel_out = None
    if general:
        sel_out = nc.dram_tensor("sel", [P, NSEG], F32,
                                 kind="ExternalOutput").ap()

    with tile.TileContext(nc) as tc:
        with ExitStack() as ctx:
            _kernel_body(ctx, tc, p1, p2, tbl, sbc, se_out, rd_out, sel_out,
                         general)
    nc.compile()
    return nc


def _kernel_body(ctx: ExitStack, tc, p1, p2, tbl, sbc, se_out, rd_out,
                 sel_out, general):
    nc = tc.nc
    consts = ctx.enter_context(tc.tile_pool(name="consts", bufs=1))
    p2pool = ctx.enter_context(tc.tile_pool(name="p2", bufs=5))
    expp = ctx.enter_context(tc.tile_pool(name="expp", bufs=6))
    psum = ctx.enter_context(tc.tile_pool(name="psum", bufs=1, space="PSUM"))

    p2t = p2.rearrange("(p i k) c -> i p (k c)", p=P, i=NT, k=KS)
    p1h = p1.rearrange("(p h j) c -> h p (j c)", p=P, h=2, j=NSEG // 2)

    p1big = consts.tile([P, NSEG * C1], F32)
    oh_all = consts.tile([P, NSEG * C1], F32R)
    tbl_sb = consts.tile([C1, C2], F32)
    rmax = consts.tile([P, NSEG], F32)
    se_all = consts.tile([P, NSEG], F32)
    rowdots = consts.tile([C1, 2], F32)

    if general:
        sbc_sb = consts.tile([P, C1], F32)
        ss_scratch = consts.tile([P, JQ * C1], F32)
        ss3 = ss_scratch[:].rearrange("p (j c) -> p j c", j=JQ)
        sbc3 = sbc_sb[:].unsqueeze(1).broadcast_to((P, JQ, C1))
        sel_s_all = consts.tile([P, NSEG], F32)

    G_a = psum.tile([C1, C2], F32)
    G_b = psum.tile([C1, C2], F32)

    p1big3 = p1big[:].rearrange("p (j c) -> p j c", j=NSEG)
    oh3 = oh_all[:].rearrange("p (j c) -> p j c", j=NSEG)

    def onehot_quarter(q):
        js = slice(q * JQ, (q + 1) * JQ)
        nc.vector.reduce_max(rmax[:, js], p1big3[:, js, :], axis=X)
        rmx3 = rmax[:, js].unsqueeze(2).broadcast_to((P, JQ, C1))
        nc.vector.tensor_tensor(oh3[:, js, :], p1big3[:, js, :], rmx3,
                                op=ALU.is_ge)
        if general:
            nc.vector.tensor_tensor(ss3[:], oh3[:, js, :].bitcast(F32), sbc3,
                                    op=ALU.mult)
            nc.vector.reduce_sum(sel_s_all[:, js], ss3[:], axis=X)

    t2_tiles = []

    def load_tile(i):
        t2 = p2pool.tile([P, KS * C2], F32R, tag="p2")
        if i == 0 or i == NT - 1:
            for k in range(KS):
                nc.sync.dma_start(t2[:, bass.ts(k, C2)],
                                  p2t[i][:, bass.ts(k, C2)].bitcast(F32R))
        elif i >= NT - 3:
            for k in range(0, KS, 2):
                nc.sync.dma_start(t2[:, k * C2:(k + 2) * C2],
                                  p2t[i][:, k * C2:(k + 2) * C2].bitcast(F32R))
        else:
            nc.sync.dma_start(t2[:], p2t[i].bitcast(F32R))
        t2_tiles.append(t2)

    def consume_tile(i):
        t2 = t2_tiles[i]
        for k in range(KS):
            seg = i * KS + k
            se_col = se_all[:, seg:seg + 1]
            et = expp.tile([P, C2], F32, tag="exp1")
            if seg in ACT_ACCUM_SEGS:
                nc.scalar.activation(et[:], t2[:, bass.ts(k, C2)].bitcast(F32),
                                     ACTF.Exp, accum_out=se_col)
            else:
                nc.scalar.activation(et[:], t2[:, bass.ts(k, C2)].bitcast(F32),
                                     ACTF.Exp)
                nc.vector.reduce_sum(se_col, et[:], axis=X)
        G = G_a if i <= GA_LAST else G_b
        for k in range(KS):
            seg = i * KS + k
            for lo, hi in CHUNKS:
                nc.tensor.matmul(G[:, lo:hi], oh_all[:, bass.ts(seg, C1)],
                                 t2[:, k * C2 + lo:k * C2 + hi],
                                 start=(k == 0 and i in (0, GA_LAST + 1)),
                                 stop=(k == KS - 1 and i in (GA_LAST, NT - 1)))

    nc.sync.dma_start(p1big[:, 0:NSEG * C1 // 2], p1h[0])
    load_tile(0)
    load_tile(1)
    nc.sync.dma_start(p1big[:, NSEG * C1 // 2:], p1h[1])
    load_tile(2)
    nc.sync.dma_start(tbl_sb[:], tbl[:, :])
    if general:
        nc.sync.dma_start(sbc_sb[:], sbc[:, :])
    for i in range(3, NT):
        load_tile(i)

    gt_scratch = consts.tile([C1, C2], F32)
    for i in range(NT):
        if i % 4 == 0:
            onehot_quarter(i // 4)
        consume_tile(i)
        if i == GA_LAST + 1:
            nc.vector.tensor_mul(gt_scratch[:], G_a[:], tbl_sb[:])
            nc.vector.tensor_reduce(rowdots[:, 0:1], gt_scratch[:], axis=X,
                                    op=ALU.add, negate=True)

    nc.vector.tensor_mul(gt_scratch[:], G_b[:], tbl_sb[:])
    nc.vector.tensor_reduce(rowdots[:, 1:2], gt_scratch[:], axis=X,
                            op=ALU.add, negate=True)
    nc.sync.dma_start(se_out[:, :], se_all[:])
    if general:
        nc.sync.dma_start(sel_out[:, :], sel_s_all[:])
    nc.sync.dma_start(rd_out[:, :], rowdots[:])


_PROGRAM_CACHE: dict = {}


def _program(mode: str = "fast2") -> bass.Bass:
    if mode not in _PROGRAM_CACHE:
        if mode == "fast2":
            _PROGRAM_CACHE[mode] = _build_fast2()
        else:
            _PROGRAM_CACHE[mode] = _build_program(mode == "general")
    return _PROGRAM_CACHE[mode]


def _row_sums(table):
    return np.asarray(table, dtype=np.float32).sum(axis=1, dtype=np.float32)


def _is_uniform_s(s):
    return bool(np.all(np.abs(s - s[0]) <= 1e-6 * max(1.0, abs(float(s[0])))))


def _block_uniform_u(table):
    """u if table is the uniform block table (t[c, c*B:(c+1)*B] = u), else
    None."""
    t = np.ascontiguousarray(table, dtype=np.float32)
    if t.shape != (C1, C2):
        return None
    u = float(t[0, 0])
    if u == 0.0:
        return None
    expect = np.zeros((C1, C2), dtype=np.float32)
    for c in range(C1):
        expect[c, c * BLOCK:(c + 1) * BLOCK] = np.float32(u)
    return u if np.array_equal(t, expect) else None


def _in_maps(pred1_logits, pred2_logits, table, mode: str):
    p1 = np.ascontiguousarray(pred1_logits, dtype=np.float32)
    p2 = np.ascontiguousarray(pred2_logits, dtype=np.float32)
    tbl = np.ascontiguousarray(table, dtype=np.float32)
    maps = []
    for k in range(NCORES):
        m = {
            "p1": np.ascontiguousarray(p1[k * BC:(k + 1) * BC]),
            "p2": np.ascontiguousarray(p2[k * BC:(k + 1) * BC]),
        }
        if mode != "fast2":
            m["tbl"] = tbl
        if mode == "general":
            s = _row_sums(tbl)
            m["sbc"] = np.ascontiguousarray(np.tile(s, (P, 1)))
        maps.append(m)
    return maps


def _combine_fast2(result, u):
    sd = np.asarray(result["sd"], dtype=np.float64)
    se = sd[:, :NSEG]
    dot = sd[:, NSEG:NSEG + ND]
    return (BLOCK * u) * np.log(se).sum() - u * dot.sum()


def _combine(result, s0, general):
    lse = np.log(np.asarray(result["se"], dtype=np.float64))
    if general:
        lse = lse * np.asarray(result["sel"], dtype=np.float64)
        lse_term = lse.sum()
    else:
        lse_term = s0 * lse.sum()
    return lse_term + np.asarray(result["rd"], dtype=np.float64).sum()


def run_on_device(pred1_logits, pred2_logits, table, **spmd_kwargs):
    """Compile/run the SPMD program on cores 0-7; returns (loss, results)."""
    u = _block_uniform_u(table)
    if u is not None:
        mode = "fast2"
    else:
        s = _row_sums(table)
        mode = "fast" if _is_uniform_s(s) else "general"
    nc = _program(mode)
    res = run_bass_kernel_spmd(
        nc, _in_maps(pred1_logits, pred2_logits, table, mode),
        core_ids=list(range(NCORES)), **spmd_kwargs)
    if mode == "fast2":
        partials = [_combine_fast2(r, u) for r in res.results]
    else:
        s0 = np.float64(_row_sums(table)[0])
        partials = [_combine(r, s0, mode == "general") for r in res.results]
    loss = np.float32(np.sum(partials, dtype=np.float64) / B)
    return np.asarray(loss), res


def kernel(pred1_logits, pred2_logits, table):
    loss, _ = run_on_device(pred1_logits, pred2_logits, table)
    return loss


# revision 25
# speedup vs baseline: 1.0980x; 1.0276x over previous
"""Trainium2 Bass kernel for the consistency-loss problem.

loss = -mean_b( table[argmax_c pred1[b,c]] . log_softmax(pred2[b]) )

Fast path ("fast2") exploits the block structure of the harness table
(table[c, c*10:(c+1)*10] = u, zeros elsewhere, same u for every row):

    loss_b = BLOCK*u * lse_b - u * S[b, c*_b]
    lse_b  = log(sum_j exp(pred2[b,j]))
    S[b,c] = sum of the 10-wide block c of pred2 row b

Per 128x1000 segment the device computes only two row scalars:
 - se  = sum_j exp(pred2)       (ACT Exp, row-sum on the ACT accumulator
                                 or a fused Pool op)
 - dot = sum_j pred2 * mask     (one fused multiply+row-reduce
                                 scalar_tensor_tensor; mask is the argmax
                                 one-hot broadcast 10-wide via a stride-0
                                 view, so no PE matmul and no table on
                                 device at all)
Both ship as one [128, 128] tile; the host applies log/scales in f64.

Engine budget per core (64 segments, ~86us DMA stream @ ~425 GB/s):
 - ACT: 64 Exp (~69us) + 40 accumulator reads (~10us)
 - Pool (otherwise idle!): 24 se row-sums + 12 dot jobs (~56us)
 - DVE: one-hot quarters (~14us) + 52 dot jobs (~61us)
 - PE: unused.  All engines < DMA stream -> DMA-bound.
DMA: tile0 + pred1 ride the SWDGE (gpsimd) queue starting ~3us, before
the sync-ring preamble finishes (~8us); tiles 1..15 own the sync ring.
Tail tiles split finer so the last segments land continuously.

Sharding: data-parallel over B across 8 NeuronCores; host combines the
per-core partial outputs.  Non-block tables fall back to the previous
matmul-based programs ("fast"/"general"), which handle any table.
"""

import sys
from contextlib import ExitStack

import numpy as np

for _p in ("/opt/trn_rl_repo", "/root/.axon_site/_ro/trn_rl_repo"):
    if _p not in sys.path:
        sys.path.append(_p)

import concourse.bass as bass
import concourse.tile as tile
from concourse import bacc, mybir
from concourse.bass_utils import run_bass_kernel_spmd

B, C1, C2 = 65536, 100, 1000
BLOCK = C2 // C1            # 10 fine classes per coarse class
NCORES = 8
BC = B // NCORES            # rows per core (8192)
P = 128                     # partitions
KS = 4                      # sub-rows per partition per tile
NT = BC // (P * KS)         # tiles per core (16)
NSEG = BC // P              # per-row segments per core (64) == NT*KS
NQ = 4                      # one-hot quarters (DVE op granularity)
JQ = NSEG // NQ             # segments per quarter (16)
F32 = mybir.dt.float32
F32R = mybir.dt.float32r
X = mybir.AxisListType.X
ALU = mybir.AluOpType
ACTF = mybir.ActivationFunctionType

# ---- fast2 static knobs ----------------------------------------------------
# early tiles whose dot term accumulates on the (otherwise idle) PE as
# G += onehot^T @ pred2, folded once against the static block mask; late
# tiles use the fused DVE dot job (PE would fall behind the stream there).
# Must be a prefix range (dot-column layout assumes it).
N_PE_TILES = 8
PE_D_TILES = frozenset(range(N_PE_TILES))
N_PE_SEG = KS * N_PE_TILES
# dot-term columns: one per DVE-dot segment
ND = NSEG - N_PE_SEG
# per-tile row-sum split: which sub-rows k reduce on the DVE (the rest ride
# the ACT accumulator).  Tail tiles keep everything on ACT (cheap reads).
DVE_R_KS = {i: ((0, 2) if i < 12 else ()) for i in range(NT)}
PSUM_CHUNKS = [(0, 512), (512, C2)]


def _build_fast2() -> bass.Bass:
    nc = bacc.Bacc("TRN2", target_bir_lowering=False, debug=False,
                   num_devices=NCORES)
    p1 = nc.dram_tensor("p1", [BC, C1], F32, kind="ExternalInput").ap()
    p2 = nc.dram_tensor("p2", [BC, C2], F32, kind="ExternalInput").ap()
    bm = nc.dram_tensor("bm", [C1, C2], F32, kind="ExternalInput").ap()
    sd_out = nc.dram_tensor("sd", [P, NSEG + ND], F32,
                            kind="ExternalOutput").ap()
    gd_out = nc.dram_tensor("gd", [C1, 1], F32, kind="ExternalOutput").ap()

    with tile.TileContext(nc) as tc:
        with ExitStack() as ctx:
            _fast2_body(ctx, tc, p1, p2, bm, sd_out, gd_out)
    nc.compile()
    return nc


def _fast2_body(ctx: ExitStack, tc, p1, p2, bm, sd_out, gd_out):
    nc = tc.nc
    consts = ctx.enter_context(tc.tile_pool(name="consts", bufs=1))
    p2pool = ctx.enter_context(tc.tile_pool(name="p2", bufs=6))
    etp = ctx.enter_context(tc.tile_pool(name="expp", bufs=4))
    psum = ctx.enter_context(tc.tile_pool(name="psum", bufs=1, space="PSUM"))

    # row (p*64 + i*4 + k)  <->  tile i, partition p, sub-row k
    p2t = p2.rearrange("(p i k) c -> i p (k c)", p=P, i=NT, k=KS)
    # pred1 quarters: per-partition 6.4KB contiguous runs, arriving just
    # ahead of the segments whose row-max they feed
    p1q = p1.rearrange("(p q j) c -> q p (j c)", p=P, q=NQ, j=JQ)

    p1big = consts.tile([P, NSEG * C1], F32)
    oh_pe = consts.tile([P, N_PE_SEG * C1], F32R)   # one-hot for PE tiles
    rmax = consts.tile([P, NSEG], F32)
    sd_all = consts.tile([P, NSEG + ND], F32)
    scr = consts.tile([P, C2], F32)
    bm_sb = consts.tile([C1, C2], F32)
    gscr = consts.tile([C1, C2], F32)
    gdot = consts.tile([C1, 1], F32)
    G = psum.tile([C1, C2], F32)

    p1big3 = p1big[:].rearrange("p (j c) -> p j c", j=NSEG)
    oh3 = oh_pe[:].rearrange("p (j c) -> p j c", j=N_PE_SEG)
    scr3 = scr[:].rearrange("p (c b) -> p c b", b=BLOCK)

    def rmax_quarter(q):
        js = slice(q * JQ, (q + 1) * JQ)
        nc.vector.reduce_max(rmax[:, js], p1big3[:, js, :], axis=X)
        if q * JQ < N_PE_SEG:
            # one-hot for the PE tiles' segments (f32r bits for LDWEIGHTS)
            rmx3 = rmax[:, js].unsqueeze(2).broadcast_to((P, JQ, C1))
            nc.vector.tensor_tensor(oh3[:, js, :], p1big3[:, js, :], rmx3,
                                    op=ALU.is_ge)

    t2_tiles = []

    def load_tile(i, eng):
        t2 = p2pool.tile([P, KS * C2], F32R, tag="p2")
        if i == 0 or i == NT - 1:
            # k-split: pipeline head starts early / tail lands continuously
            for k in range(KS):
                eng.dma_start(t2[:, bass.ts(k, C2)],
                              p2t[i][:, bass.ts(k, C2)].bitcast(F32R))
        elif i >= NT - 3:
            # pair-split taper for the tiles feeding the tail
            for k in range(0, KS, 2):
                eng.dma_start(t2[:, k * C2:(k + 2) * C2],
                              p2t[i][:, k * C2:(k + 2) * C2].bitcast(F32R))
        else:
            eng.dma_start(t2[:], p2t[i].bitcast(F32R))
        t2_tiles.append(t2)

    def consume_tile(i):
        t2 = t2_tiles[i]
        for k in range(KS):
            seg = i * KS + k
            t2f = t2[:, bass.ts(k, C2)].bitcast(F32)
            se_col = sd_all[:, seg:seg + 1]
            if i not in PE_D_TILES:
                # fused dot job on the DVE: (pred1_bcast >= rmax) * pred2,
                # row-reduced into the accumulator — the one-hot never
                # materializes; 1 instr / segment.
                dot_col = sd_all[:, NSEG + seg - N_PE_SEG:
                                 NSEG + seg - N_PE_SEG + 1]
                p1b = p1big3[:, seg, :].unsqueeze(2).broadcast_to(
                    (P, C1, BLOCK))
                t2s = t2f.rearrange("p (c b) -> p c b", b=BLOCK)
                nc.vector.scalar_tensor_tensor(
                    scr3, p1b, rmax[:, seg:seg + 1], t2s,
                    op0=ALU.is_ge, op1=ALU.mult, accum_out=dot_col)
            et = etp.tile([P, C2], F32, tag="exp1")
            if k in DVE_R_KS[i]:
                nc.scalar.activation(et[:], t2f, ACTF.Exp)
                nc.vector.reduce_sum(se_col, et[:], axis=X)
            else:
                nc.scalar.activation(et[:], t2f, ACTF.Exp, accum_out=se_col)
        if i in PE_D_TILES:
            # dot term via the PE: G += onehot^T @ pred2 (PSUM accumulate)
            for k in range(KS):
                seg = i * KS + k
                for lo, hi in PSUM_CHUNKS:
                    nc.tensor.matmul(
                        G[:, lo:hi], oh_pe[:, bass.ts(seg, C1)],
                        t2[:, k * C2 + lo:k * C2 + hi],
                        start=(k == 0 and i == min(PE_D_TILES)),
                        stop=(k == KS - 1 and i == max(PE_D_TILES)))

    # --- DMA schedule (single sync ring, priority order) ---
    t0 = p2pool.tile([P, KS * C2], F32R, tag="p2")
    t2_tiles.append(t0)
    nc.sync.dma_start(t0[:, 0:C2], p2t[0][:, 0:C2].bitcast(F32R))
    nc.sync.dma_start(p1big[:, 0:JQ * C1], p1q[0])
    for k in range(1, KS):
        nc.sync.dma_start(t0[:, bass.ts(k, C2)],
                          p2t[0][:, bass.ts(k, C2)].bitcast(F32R))
    for i in range(1, NT):
        load_tile(i, nc.sync)
        if i <= 3:
            nc.sync.dma_start(p1big[:, i * JQ * C1:(i + 1) * JQ * C1], p1q[i])
        if i == 4:
            nc.sync.dma_start(bm_sb[:], bm[:, :])

    for i in range(NT):
        if i % 4 == 0:
            rmax_quarter(i // 4)
        consume_tile(i)
        if i == max(PE_D_TILES) + 1:
            # G complete after the last PE tile; fold against the block mask
            # (DVE STT reads PSUM directly) and ship mid-stream
            nc.vector.scalar_tensor_tensor(
                gscr[:], G[:], 1.0, bm_sb[:], op0=ALU.mult, op1=ALU.mult,
                accum_out=gdot[:])
            nc.sync.dma_start(gd_out[:, :], gdot[:])

    nc.sync.dma_start(sd_out[:, :], sd_all[:])


# ===========================================================================
# Fallback programs (arbitrary tables): previous matmul-based kernel.
# ===========================================================================

CHUNKS = [(0, 512), (512, C2)]
GA_LAST = NT - 4
ACT_ACCUM_SEGS = frozenset(s for s in range(4, 56, 3)) | {NSEG - 2, NSEG - 1}


def _build_program(general: bool) -> bass.Bass:
    nc = bacc.Bacc("TRN2", target_bir_lowering=False, debug=False,
                   num_devices=NCORES)
    p1 = nc.dram_tensor("p1", [BC, C1], F32, kind="ExternalInput").ap()
    p2 = nc.dram_tensor("p2", [BC, C2], F32, kind="ExternalInput").ap()
    tbl = nc.dram_tensor("tbl", [C1, C2], F32, kind="ExternalInput").ap()
    sbc = None
    if general:
        sbc = nc.dram_tensor("sbc", [P, C1], F32, kind="ExternalInput").ap()
    se_out = nc.dram_tensor("se", [P, NSEG], F32, kind="ExternalOutput").ap()
    rd_out = nc.dram_tensor("rd", [C1, 2], F32, kind="ExternalOutput").ap()
    sel_out = None
    if general:
        sel_out = nc.dram_tensor("sel", [P, NSEG], F32,
                                 kind="ExternalOutput").ap()

    with tile.TileContext(nc) as tc:
        with ExitStack() as ctx:
            _kernel_body(ctx, tc, p1, p2, tbl, sbc, se_out, rd_out, sel_out,
                         general)
    nc.compile()
    return nc


def _kernel_body(ctx: ExitStack, tc, p1, p2, tbl, sbc, se_out, rd_out,
                 sel_out, general):
    nc = tc.nc
    consts = ctx.enter_context(tc.tile_pool(name="consts", bufs=1))
    p2pool = ctx.enter_context(tc.tile_pool(name="p2", bufs=5))
    expp = ctx.enter_context(tc.tile_pool(name="expp", bufs=6))
    psum = ctx.enter_context(tc.tile_pool(name="psum", bufs=1, space="PSUM"))

    p2t = p2.rearrange("(p i k) c -> i p (k c)", p=P, i=NT, k=KS)
    p1h = p1.rearrange("(p h j) c -> h p (j c)", p=P, h=2, j=NSEG // 2)

    p1big = consts.tile([P, NSEG * C1], F32)
    oh_all = consts.tile([P, NSEG * C1], F32R)
    tbl_sb = consts.tile([C1, C2], F32)
    rmax = consts.tile([P, NSEG], F32)
    se_all = consts.tile([P, NSEG], F32)
    rowdots = consts.tile([C1, 2], F32)

    if general:
        sbc_sb = consts.tile([P, C1], F32)
        ss_scratch = consts.tile([P, JQ * C1], F32)
        ss3 = ss_scratch[:].rearrange("p (j c) -> p j c", j=JQ)
        sbc3 = sbc_sb[:].unsqueeze(1).broadcast_to((P, JQ, C1))
        sel_s_all = consts.tile([P, NSEG], F32)

    G_a = psum.tile([C1, C2], F32)
    G_b = psum.tile([C1, C2], F32)

    p1big3 = p1big[:].rearrange("p (j c) -> p j c", j=NSEG)
    oh3 = oh_all[:].rearrange("p (j c) -> p j c", j=NSEG)

    def onehot_quarter(q):
        js = slice(q * JQ, (q + 1) * JQ)
        nc.vector.reduce_max(rmax[:, js], p1big3[:, js, :], axis=X)
        rmx3 = rmax[:, js].unsqueeze(2).broadcast_to((P, JQ, C1))
        nc.vector.tensor_tensor(oh3[:, js, :], p1big3[:, js, :], rmx3,
                                op=ALU.is_ge)
        if general:
            nc.vector.tensor_tensor(ss3[:], oh3[:, js, :].bitcast(F32), sbc3,
                                    op=ALU.mult)
            nc.vector.reduce_sum(sel_s_all[:, js], ss3[:], axis=X)

    t2_tiles = []

    def load_tile(i):
        t2 = p2pool.tile([P, KS * C2], F32R, tag="p2")
        if i == 0 or i == NT - 1:
            for k in range(KS):
                nc.sync.dma_start(t2[:, bass.ts(k, C2)],
                                  p2t[i][:, bass.ts(k, C2)].bitcast(F32R))
        elif i >= NT - 3:
            for k in range(0, KS, 2):
                nc.sync.dma_start(t2[:, k * C2:(k + 2) * C2],
                                  p2t[i][:, k * C2:(k + 2) * C2].bitcast(F32R))
        else:
            nc.sync.dma_start(t2[:], p2t[i].bitcast(F32R))
        t2_tiles.append(t2)

    def consume_tile(i):
        t2 = t2_tiles[i]
        for k in range(KS):
            seg = i * KS + k
            se_col = se_all[:, seg:seg + 1]
            et = expp.tile([P, C2], F32, tag="exp1")
            if seg in ACT_ACCUM_SEGS:
                nc.scalar.activation(et[:], t2[:, bass.ts(k, C2)].bitcast(F32),
                                     ACTF.Exp, accum_out=se_col)
            else:
                nc.scalar.activation(et[:], t2[:, bass.ts(k, C2)].bitcast(F32),
                                     ACTF.Exp)
                nc.vector.reduce_sum(se_col, et[:], axis=X)
        G = G_a if i <= GA_LAST else G_b
        for k in range(KS):
            seg = i * KS + k
            for lo, hi in CHUNKS:
                nc.tensor.matmul(G[:, lo:hi], oh_all[:, bass.ts(seg, C1)],
                                 t2[:, k * C2 + lo:k * C2 + hi],
                                 start=(k == 0 and i in (0, GA_LAST + 1)),
                                 stop=(k == KS - 1 and i in (GA_LAST, NT - 1)))

    nc.sync.dma_start(p1big[:, 0:NSEG * C1 // 2], p1h[0])
    load_tile(0)
    load_tile(1)
    nc.sync.dma_start(p1big[:, NSEG * C1 // 2:], p1h[1])
    load_tile(2)
    nc.sync.dma_start(tbl_sb[:], tbl[:, :])
    if general:
        nc.sync.dma_start(sbc_sb[:], sbc[:, :])
    for i in range(3, NT):
        load_tile(i)

    gt_scratch = consts.tile([C1, C2], F32)
    for i in range(NT):
        if i % 4 == 0:
            onehot_quarter(i // 4)
        consume_tile(i)
        if i == GA_LAST + 1:
            nc.vector.tensor_mul(gt_scratch[:], G_a[:], tbl_sb[:])
            nc.vector.tensor_reduce(rowdots[:, 0:1], gt_scratch[:], axis=X,
                                    op=ALU.add, negate=True)

    nc.vector.tensor_mul(gt_scratch[:], G_b[:], tbl_sb[:])
    nc.vector.tensor_reduce(rowdots[:, 1:2], gt_scratch[:], axis=X,
                            op=ALU.add, negate=True)
    nc.sync.dma_start(se_out[:, :], se_all[:])
    if general:
        nc.sync.dma_start(sel_out[:, :], sel_s_all[:])
    nc.sync.dma_start(rd_out[:, :], rowdots[:])


_PROGRAM_CACHE: dict = {}


def _program(mode: str = "fast2") -> bass.Bass:
    if mode not in _PROGRAM_CACHE:
        if mode == "fast2":
            _PROGRAM_CACHE[mode] = _build_fast2()
        else:
            _PROGRAM_CACHE[mode] = _build_program(mode == "general")
    return _PROGRAM_CACHE[mode]


def _row_sums(table):
    return np.asarray(table, dtype=np.float32).sum(axis=1, dtype=np.float32)


def _is_uniform_s(s):
    return bool(np.all(np.abs(s - s[0]) <= 1e-6 * max(1.0, abs(float(s[0])))))


def _block_uniform_u(table):
    """u if table is the uniform block table (t[c, c*B:(c+1)*B] = u), else
    None."""
    t = np.ascontiguousarray(table, dtype=np.float32)
    if t.shape != (C1, C2):
        return None
    u = float(t[0, 0])
    if u == 0.0:
        return None
    expect = np.zeros((C1, C2), dtype=np.float32)
    for c in range(C1):
        expect[c, c * BLOCK:(c + 1) * BLOCK] = np.float32(u)
    return u if np.array_equal(t, expect) else None


def _block_mask():
    bm = np.zeros((C1, C2), dtype=np.float32)
    for c in range(C1):
        bm[c, c * BLOCK:(c + 1) * BLOCK] = 1.0
    return bm


def _in_maps(pred1_logits, pred2_logits, table, mode: str):
    p1 = np.ascontiguousarray(pred1_logits, dtype=np.float32)
    p2 = np.ascontiguousarray(pred2_logits, dtype=np.float32)
    tbl = np.ascontiguousarray(table, dtype=np.float32)
    bm = _block_mask() if mode == "fast2" else None
    maps = []
    for k in range(NCORES):
        m = {
            "p1": np.ascontiguousarray(p1[k * BC:(k + 1) * BC]),
            "p2": np.ascontiguousarray(p2[k * BC:(k + 1) * BC]),
        }
        if mode == "fast2":
            m["bm"] = bm
        else:
            m["tbl"] = tbl
        if mode == "general":
            s = _row_sums(tbl)
            m["sbc"] = np.ascontiguousarray(np.tile(s, (P, 1)))
        maps.append(m)
    return maps


def _combine_fast2(result, u):
    sd = np.asarray(result["sd"], dtype=np.float64)
    se = sd[:, :NSEG]
    dot = sd[:, NSEG:].sum()
    dot += np.asarray(result["gd"], dtype=np.float64).sum()
    return (BLOCK * u) * np.log(se).sum() - u * dot


def _combine(result, s0, general):
    lse = np.log(np.asarray(result["se"], dtype=np.float64))
    if general:
        lse = lse * np.asarray(result["sel"], dtype=np.float64)
        lse_term = lse.sum()
    else:
        lse_term = s0 * lse.sum()
    return lse_term + np.asarray(result["rd"], dtype=np.float64).sum()


def run_on_device(pred1_logits, pred2_logits, table, **spmd_kwargs):
    """Compile/run the SPMD program on cores 0-7; returns (loss, results)."""
    u = _block_uniform_u(table)
    if u is not None:
        mode = "fast2"
    else:
        s = _row_sums(table)
        mode = "fast" if _is_uniform_s(s) else "general"
    nc = _program(mode)
    res = run_bass_kernel_spmd(
        nc, _in_maps(pred1_logits, pred2_logits, table, mode),
        core_ids=list(range(NCORES)), **spmd_kwargs)
    if mode == "fast2":
        partials = [_combine_fast2(r, u) for r in res.results]
    else:
        s0 = np.float64(_row_sums(table)[0])
        partials = [_combine(r, s0, mode == "general") for r in res.results]
    loss = np.float32(np.sum(partials, dtype=np.float64) / B)
    return np.asarray(loss), res


def kernel(pred1_logits, pred2_logits, table):
    loss, _ = run_on_device(pred1_logits, pred2_logits, table)
    return loss


# revision 27
# speedup vs baseline: 1.1765x; 1.0715x over previous
"""Trainium2 Bass kernel for the consistency-loss problem.

loss = -mean_b( table[argmax_c pred1[b,c]] . log_softmax(pred2[b]) )

Fast path ("fast2") exploits the block structure of the harness table
(table[c, c*10:(c+1)*10] = u, zeros elsewhere, same u for every row):

    loss_b = BLOCK*u * lse_b - u * S[b, c*_b]
    lse_b  = log(sum_j exp(pred2[b,j]))
    S[b,c] = sum of the 10-wide block c of pred2 row b

Per 128x1000 segment the device computes only two row scalars:
 - se  = sum_j exp(pred2)       (ACT Exp, row-sum on the ACT accumulator
                                 or a fused Pool op)
 - dot = sum_j pred2 * mask     (one fused multiply+row-reduce
                                 scalar_tensor_tensor; mask is the argmax
                                 one-hot broadcast 10-wide via a stride-0
                                 view, so no PE matmul and no table on
                                 device at all)
Both ship as one [128, 128] tile; the host applies log/scales in f64.

Engine budget per core (64 segments, ~86us DMA stream @ ~425 GB/s):
 - ACT: 64 Exp (~69us) + 40 accumulator reads (~10us)
 - Pool (otherwise idle!): 24 se row-sums + 12 dot jobs (~56us)
 - DVE: one-hot quarters (~14us) + 52 dot jobs (~61us)
 - PE: unused.  All engines < DMA stream -> DMA-bound.
DMA: tile0 + pred1 ride the SWDGE (gpsimd) queue starting ~3us, before
the sync-ring preamble finishes (~8us); tiles 1..15 own the sync ring.
Tail tiles split finer so the last segments land continuously.

Sharding: data-parallel over B across 8 NeuronCores; host combines the
per-core partial outputs.  Non-block tables fall back to the previous
matmul-based programs ("fast"/"general"), which handle any table.
"""

import sys
from contextlib import ExitStack

import numpy as np

for _p in ("/opt/trn_rl_repo", "/root/.axon_site/_ro/trn_rl_repo"):
    if _p not in sys.path:
        sys.path.append(_p)

import concourse.bass as bass
import concourse.tile as tile
from concourse import bacc, mybir
from concourse.bass_utils import run_bass_kernel_spmd

B, C1, C2 = 65536, 100, 1000
BLOCK = C2 // C1            # 10 fine classes per coarse class
NCORES = 8
BC = B // NCORES            # rows per core (8192)
P = 128                     # partitions
KS = 4                      # sub-rows per partition per tile
NT = BC // (P * KS)         # tiles per core (16)
NSEG = BC // P              # per-row segments per core (64) == NT*KS
NQ = 4                      # one-hot quarters (DVE op granularity)
JQ = NSEG // NQ             # segments per quarter (16)
F32 = mybir.dt.float32
F32R = mybir.dt.float32r
X = mybir.AxisListType.X
ALU = mybir.AluOpType
ACTF = mybir.ActivationFunctionType

# ---- fast2 static knobs ----------------------------------------------------
# early tiles whose dot term accumulates on the (otherwise idle) PE as
# G += onehot^T @ pred2, folded once against the static block mask; late
# tiles use the fused DVE dot job (PE would fall behind the stream there).
# Must be a prefix range (dot-column layout assumes it).
N_PE_TILES = 10
PE_D_TILES = frozenset(range(N_PE_TILES))
N_PE_SEG = KS * N_PE_TILES
# dot-term columns: one per DVE-dot segment
ND = NSEG - N_PE_SEG
# per-tile row-sum split: which sub-rows k reduce on the DVE (the rest ride
# the ACT accumulator).  DVE row-sums sit in its idle early window (the PE
# tiles); the DVE-dot tiles keep everything on ACT (cheap reads).
DVE_R_KS = {i: ((0, 1, 2) if i in PE_D_TILES else ()) for i in range(NT)}
PSUM_CHUNKS = [(0, 512), (512, C2)]


def _build_fast2() -> bass.Bass:
    nc = bacc.Bacc("TRN2", target_bir_lowering=False, debug=False,
                   num_devices=NCORES)
    p1 = nc.dram_tensor("p1", [BC, C1], F32, kind="ExternalInput").ap()
    p2 = nc.dram_tensor("p2", [BC, C2], F32, kind="ExternalInput").ap()
    bm = nc.dram_tensor("bm", [C1, C2], F32, kind="ExternalInput").ap()
    sd_out = nc.dram_tensor("sd", [P, NSEG + ND], F32,
                            kind="ExternalOutput").ap()
    gd_out = nc.dram_tensor("gd", [C1, 1], F32, kind="ExternalOutput").ap()

    with tile.TileContext(nc) as tc:
        with ExitStack() as ctx:
            _fast2_body(ctx, tc, p1, p2, bm, sd_out, gd_out)
    nc.compile()
    return nc


def _fast2_body(ctx: ExitStack, tc, p1, p2, bm, sd_out, gd_out):
    nc = tc.nc
    consts = ctx.enter_context(tc.tile_pool(name="consts", bufs=1))
    p2pool = ctx.enter_context(tc.tile_pool(name="p2", bufs=6))
    etp = ctx.enter_context(tc.tile_pool(name="expp", bufs=4))
    psum = ctx.enter_context(tc.tile_pool(name="psum", bufs=1, space="PSUM"))

    # row (p*64 + i*4 + k)  <->  tile i, partition p, sub-row k
    p2t = p2.rearrange("(p i k) c -> i p (k c)", p=P, i=NT, k=KS)
    # pred1 quarters: per-partition 6.4KB contiguous runs, arriving just
    # ahead of the segments whose row-max they feed
    p1q = p1.rearrange("(p q j) c -> q p (j c)", p=P, q=NQ, j=JQ)

    p1big = consts.tile([P, NSEG * C1], F32)
    oh_pe = consts.tile([P, N_PE_SEG * C1], F32R)   # one-hot for PE tiles
    rmax = consts.tile([P, NSEG], F32)
    sd_all = consts.tile([P, NSEG + ND], F32)
    scr = consts.tile([P, C2], F32)
    bm_sb = consts.tile([C1, C2], F32)
    gscr = consts.tile([C1, C2], F32)
    gdot = consts.tile([C1, 1], F32)
    G = psum.tile([C1, C2], F32)

    p1big3 = p1big[:].rearrange("p (j c) -> p j c", j=NSEG)
    oh3 = oh_pe[:].rearrange("p (j c) -> p j c", j=N_PE_SEG)
    scr3 = scr[:].rearrange("p (c b) -> p c b", b=BLOCK)

    def rmax_quarter(q):
        js = slice(q * JQ, (q + 1) * JQ)
        nc.vector.reduce_max(rmax[:, js], p1big3[:, js, :], axis=X)
        if q * JQ < N_PE_SEG:
            # one-hot for the PE tiles' segments (f32r bits for LDWEIGHTS)
            jhi = min((q + 1) * JQ, N_PE_SEG)
            jso = slice(q * JQ, jhi)
            nj = jhi - q * JQ
            rmx3 = rmax[:, jso].unsqueeze(2).broadcast_to((P, nj, C1))
            nc.vector.tensor_tensor(oh3[:, jso, :], p1big3[:, jso, :], rmx3,
                                    op=ALU.is_ge)

    t2_tiles = []

    def load_tile(i, eng):
        t2 = p2pool.tile([P, KS * C2], F32R, tag="p2")
        if i == 0 or i == NT - 1:
            # k-split: pipeline head starts early / tail lands continuously
            for k in range(KS):
                eng.dma_start(t2[:, bass.ts(k, C2)],
                              p2t[i][:, bass.ts(k, C2)].bitcast(F32R))
        elif i >= NT - 3:
            # pair-split taper for the tiles feeding the tail
            for k in range(0, KS, 2):
                eng.dma_start(t2[:, k * C2:(k + 2) * C2],
                              p2t[i][:, k * C2:(k + 2) * C2].bitcast(F32R))
        else:
            eng.dma_start(t2[:], p2t[i].bitcast(F32R))
        t2_tiles.append(t2)

    def consume_tile(i):
        t2 = t2_tiles[i]
        for k in range(KS):
            seg = i * KS + k
            t2f = t2[:, bass.ts(k, C2)].bitcast(F32)
            se_col = sd_all[:, seg:seg + 1]
            if i not in PE_D_TILES:
                # fused dot job on the DVE: (pred1_bcast >= rmax) * pred2,
                # row-reduced into the accumulator — the one-hot never
                # materializes; 1 instr / segment.
                dot_col = sd_all[:, NSEG + seg - N_PE_SEG:
                                 NSEG + seg - N_PE_SEG + 1]
                p1b = p1big3[:, seg, :].unsqueeze(2).broadcast_to(
                    (P, C1, BLOCK))
                t2s = t2f.rearrange("p (c b) -> p c b", b=BLOCK)
                nc.vector.scalar_tensor_tensor(
                    scr3, p1b, rmax[:, seg:seg + 1], t2s,
                    op0=ALU.is_ge, op1=ALU.mult, accum_out=dot_col)
            et = etp.tile([P, C2], F32, tag="exp1")
            if k in DVE_R_KS[i]:
                nc.scalar.activation(et[:], t2f, ACTF.Exp)
                nc.vector.reduce_sum(se_col, et[:], axis=X)
            else:
                nc.scalar.activation(et[:], t2f, ACTF.Exp, accum_out=se_col)
        if i in PE_D_TILES:
            # dot term via the PE: G += onehot^T @ pred2 (PSUM accumulate)
            for k in range(KS):
                seg = i * KS + k
                for lo, hi in PSUM_CHUNKS:
                    nc.tensor.matmul(
                        G[:, lo:hi], oh_pe[:, bass.ts(seg, C1)],
                        t2[:, k * C2 + lo:k * C2 + hi],
                        start=(k == 0 and i == min(PE_D_TILES)),
                        stop=(k == KS - 1 and i == max(PE_D_TILES)))

    # --- DMA schedule (single sync ring, priority order) ---
    t0 = p2pool.tile([P, KS * C2], F32R, tag="p2")
    t2_tiles.append(t0)
    nc.sync.dma_start(t0[:, 0:C2], p2t[0][:, 0:C2].bitcast(F32R))
    nc.sync.dma_start(p1big[:, 0:JQ * C1], p1q[0])
    for k in range(1, KS):
        nc.sync.dma_start(t0[:, bass.ts(k, C2)],
                          p2t[0][:, bass.ts(k, C2)].bitcast(F32R))
    for i in range(1, NT):
        load_tile(i, nc.sync)
        if i <= 3:
            nc.sync.dma_start(p1big[:, i * JQ * C1:(i + 1) * JQ * C1], p1q[i])
        if i == 4:
            nc.sync.dma_start(bm_sb[:], bm[:, :])

    for i in range(NT):
        if i % 4 == 0:
            rmax_quarter(i // 4)
        consume_tile(i)
        if i == max(PE_D_TILES) + 1:
            # G complete after the last PE tile; fold against the block mask
            # (DVE STT reads PSUM directly) and ship mid-stream
            nc.vector.scalar_tensor_tensor(
                gscr[:], G[:], 1.0, bm_sb[:], op0=ALU.mult, op1=ALU.mult,
                accum_out=gdot[:])
            nc.sync.dma_start(gd_out[:, :], gdot[:])

    nc.sync.dma_start(sd_out[:, :], sd_all[:])


# ===========================================================================
# Fallback programs (arbitrary tables): previous matmul-based kernel.
# ===========================================================================

CHUNKS = [(0, 512), (512, C2)]
GA_LAST = NT - 4
ACT_ACCUM_SEGS = frozenset(s for s in range(4, 56, 3)) | {NSEG - 2, NSEG - 1}


def _build_program(general: bool) -> bass.Bass:
    nc = bacc.Bacc("TRN2", target_bir_lowering=False, debug=False,
                   num_devices=NCORES)
    p1 = nc.dram_tensor("p1", [BC, C1], F32, kind="ExternalInput").ap()
    p2 = nc.dram_tensor("p2", [BC, C2], F32, kind="ExternalInput").ap()
    tbl = nc.dram_tensor("tbl", [C1, C2], F32, kind="ExternalInput").ap()
    sbc = None
    if general:
        sbc = nc.dram_tensor("sbc", [P, C1], F32, kind="ExternalInput").ap()
    se_out = nc.dram_tensor("se", [P, NSEG], F32, kind="ExternalOutput").ap()
    rd_out = nc.dram_tensor("rd", [C1, 2], F32, kind="ExternalOutput").ap()
    sel_out = None
    if general:
        sel_out = nc.dram_tensor("sel", [P, NSEG], F32,
                                 kind="ExternalOutput").ap()

    with tile.TileContext(nc) as tc:
        with ExitStack() as ctx:
            _kernel_body(ctx, tc, p1, p2, tbl, sbc, se_out, rd_out, sel_out,
                         general)
    nc.compile()
    return nc


def _kernel_body(ctx: ExitStack, tc, p1, p2, tbl, sbc, se_out, rd_out,
                 sel_out, general):
    nc = tc.nc
    consts = ctx.enter_context(tc.tile_pool(name="consts", bufs=1))
    p2pool = ctx.enter_context(tc.tile_pool(name="p2", bufs=5))
    expp = ctx.enter_context(tc.tile_pool(name="expp", bufs=6))
    psum = ctx.enter_context(tc.tile_pool(name="psum", bufs=1, space="PSUM"))

    p2t = p2.rearrange("(p i k) c -> i p (k c)", p=P, i=NT, k=KS)
    p1h = p1.rearrange("(p h j) c -> h p (j c)", p=P, h=2, j=NSEG // 2)

    p1big = consts.tile([P, NSEG * C1], F32)
    oh_all = consts.tile([P, NSEG * C1], F32R)
    tbl_sb = consts.tile([C1, C2], F32)
    rmax = consts.tile([P, NSEG], F32)
    se_all = consts.tile([P, NSEG], F32)
    rowdots = consts.tile([C1, 2], F32)

    if general:
        sbc_sb = consts.tile([P, C1], F32)
        ss_scratch = consts.tile([P, JQ * C1], F32)
        ss3 = ss_scratch[:].rearrange("p (j c) -> p j c", j=JQ)
        sbc3 = sbc_sb[:].unsqueeze(1).broadcast_to((P, JQ, C1))
        sel_s_all = consts.tile([P, NSEG], F32)

    G_a = psum.tile([C1, C2], F32)
    G_b = psum.tile([C1, C2], F32)

    p1big3 = p1big[:].rearrange("p (j c) -> p j c", j=NSEG)
    oh3 = oh_all[:].rearrange("p (j c) -> p j c", j=NSEG)

    def onehot_quarter(q):
        js = slice(q * JQ, (q + 1) * JQ)
        nc.vector.reduce_max(rmax[:, js], p1big3[:, js, :], axis=X)
        rmx3 = rmax[:, js].unsqueeze(2).broadcast_to((P, JQ, C1))
        nc.vector.tensor_tensor(oh3[:, js, :], p1big3[:, js, :], rmx3,
                                op=ALU.is_ge)
        if general:
            nc.vector.tensor_tensor(ss3[:], oh3[:, js, :].bitcast(F32), sbc3,
                                    op=ALU.mult)
            nc.vector.reduce_sum(sel_s_all[:, js], ss3[:], axis=X)

    t2_tiles = []

    def load_tile(i):
        t2 = p2pool.tile([P, KS * C2], F32R, tag="p2")
        if i == 0 or i == NT - 1:
            for k in range(KS):
                nc.sync.dma_start(t2[:, bass.ts(k, C2)],
                                  p2t[i][:, bass.ts(k, C2)].bitcast(F32R))
        elif i >= NT - 3:
            for k in range(0, KS, 2):
                nc.sync.dma_start(t2[:, k * C2:(k + 2) * C2],
                                  p2t[i][:, k * C2:(k + 2) * C2].bitcast(F32R))
        else:
            nc.sync.dma_start(t2[:], p2t[i].bitcast(F32R))
        t2_tiles.append(t2)

    def consume_tile(i):
        t2 = t2_tiles[i]
        for k in range(KS):
            seg = i * KS + k
            se_col = se_all[:, seg:seg + 1]
            et = expp.tile([P, C2], F32, tag="exp1")
            if seg in ACT_ACCUM_SEGS:
                nc.scalar.activation(et[:], t2[:, bass.ts(k, C2)].bitcast(F32),
                                     ACTF.Exp, accum_out=se_col)
            else:
                nc.scalar.activation(et[:], t2[:, bass.ts(k, C2)].bitcast(F32),
                                     ACTF.Exp)
                nc.vector.reduce_sum(se_col, et[:], axis=X)
        G = G_a if i <= GA_LAST else G_b
        for k in range(KS):
            seg = i * KS + k
            for lo, hi in CHUNKS:
                nc.tensor.matmul(G[:, lo:hi], oh_all[:, bass.ts(seg, C1)],
                                 t2[:, k * C2 + lo:k * C2 + hi],
                                 start=(k == 0 and i in (0, GA_LAST + 1)),
                                 stop=(k == KS - 1 and i in (GA_LAST, NT - 1)))

    nc.sync.dma_start(p1big[:, 0:NSEG * C1 // 2], p1h[0])
    load_tile(0)
    load_tile(1)
    nc.sync.dma_start(p1big[:, NSEG * C1 // 2:], p1h[1])
    load_tile(2)
    nc.sync.dma_start(tbl_sb[:], tbl[:, :])
    if general:
        nc.sync.dma_start(sbc_sb[:], sbc[:, :])
    for i in range(3, NT):
        load_tile(i)

    gt_scratch = consts.tile([C1, C2], F32)
    for i in range(NT):
        if i % 4 == 0:
            onehot_quarter(i // 4)
        consume_tile(i)
        if i == GA_LAST + 1:
            nc.vector.tensor_mul(gt_scratch[:], G_a[:], tbl_sb[:])
            nc.vector.tensor_reduce(rowdots[:, 0:1], gt_scratch[:], axis=X,
                                    op=ALU.add, negate=True)

    nc.vector.tensor_mul(gt_scratch[:], G_b[:], tbl_sb[:])
    nc.vector.tensor_reduce(rowdots[:, 1:2], gt_scratch[:], axis=X,
                            op=ALU.add, negate=True)
    nc.sync.dma_start(se_out[:, :], se_all[:])
    if general:
        nc.sync.dma_start(sel_out[:, :], sel_s_all[:])
    nc.sync.dma_start(rd_out[:, :], rowdots[:])


_PROGRAM_CACHE: dict = {}


def _program(mode: str = "fast2") -> bass.Bass:
    if mode not in _PROGRAM_CACHE:
        if mode == "fast2":
            _PROGRAM_CACHE[mode] = _build_fast2()
        else:
            _PROGRAM_CACHE[mode] = _build_program(mode == "general")
    return _PROGRAM_CACHE[mode]


def _row_sums(table):
    return np.asarray(table, dtype=np.float32).sum(axis=1, dtype=np.float32)


def _is_uniform_s(s):
    return bool(np.all(np.abs(s - s[0]) <= 1e-6 * max(1.0, abs(float(s[0])))))


def _block_uniform_u(table):
    """u if table is the uniform block table (t[c, c*B:(c+1)*B] = u), else
    None."""
    t = np.ascontiguousarray(table, dtype=np.float32)
    if t.shape != (C1, C2):
        return None
    u = float(t[0, 0])
    if u == 0.0:
        return None
    expect = np.zeros((C1, C2), dtype=np.float32)
    for c in range(C1):
        expect[c, c * BLOCK:(c + 1) * BLOCK] = np.float32(u)
    return u if np.array_equal(t, expect) else None


def _block_mask():
    bm = np.zeros((C1, C2), dtype=np.float32)
    for c in range(C1):
        bm[c, c * BLOCK:(c + 1) * BLOCK] = 1.0
    return bm


def _in_maps(pred1_logits, pred2_logits, table, mode: str):
    p1 = np.ascontiguousarray(pred1_logits, dtype=np.float32)
    p2 = np.ascontiguousarray(pred2_logits, dtype=np.float32)
    tbl = np.ascontiguousarray(table, dtype=np.float32)
    bm = _block_mask() if mode == "fast2" else None
    maps = []
    for k in range(NCORES):
        m = {
            "p1": np.ascontiguousarray(p1[k * BC:(k + 1) * BC]),
            "p2": np.ascontiguousarray(p2[k * BC:(k + 1) * BC]),
        }
        if mode == "fast2":
            m["bm"] = bm
        else:
            m["tbl"] = tbl
        if mode == "general":
            s = _row_sums(tbl)
            m["sbc"] = np.ascontiguousarray(np.tile(s, (P, 1)))
        maps.append(m)
    return maps


def _combine_fast2(result, u):
    sd = np.asarray(result["sd"], dtype=np.float64)
    se = sd[:, :NSEG]
    dot = sd[:, NSEG:].sum()
    dot += np.asarray(result["gd"], dtype=np.float64).sum()
    return (BLOCK * u) * np.log(se).sum() - u * dot


def _combine(result, s0, general):
    lse = np.log(np.asarray(result["se"], dtype=np.float64))
    if general:
        lse = lse * np.asarray(result["sel"], dtype=np.float64)
        lse_term = lse.sum()
    else:
        lse_term = s0 * lse.sum()
    return lse_term + np.asarray(result["rd"], dtype=np.float64).sum()


def run_on_device(pred1_logits, pred2_logits, table, **spmd_kwargs):
    """Compile/run the SPMD program on cores 0-7; returns (loss, results)."""
    u = _block_uniform_u(table)
    if u is not None:
        mode = "fast2"
    else:
        s = _row_sums(table)
        mode = "fast" if _is_uniform_s(s) else "general"
    nc = _program(mode)
    res = run_bass_kernel_spmd(
        nc, _in_maps(pred1_logits, pred2_logits, table, mode),
        core_ids=list(range(NCORES)), **spmd_kwargs)
    if mode == "fast2":
        partials = [_combine_fast2(r, u) for r in res.results]
    else:
        s0 = np.float64(_row_sums(table)[0])
        partials = [_combine(r, s0, mode == "general") for r in res.results]
    loss = np.float32(np.sum(partials, dtype=np.float64) / B)
    return np.asarray(loss), res


def kernel(pred1_logits, pred2_logits, table):
    loss, _ = run_on_device(pred1_logits, pred2_logits, table)
    return loss
